# revision 54
# baseline (speedup 1.0000x reference)
"""GCNNet (SimpleConv sum-aggr + global_mean_pool + 2-layer MLP) on 8 trn2 cores.

Math: out[g] = MLP(relu(sums[g] / max(counts[g],1)))
  sums[g,:]  = sum_e w_e * x[src_e,:] * [batch[dst_e]==g]
  counts[g]  = #{i : batch[i]==g}

Sharding: by graph range (64 graphs per core) -> fully independent cores, no
collective.  The host canonicalizes each core's edge list like a COO->CSR
conversion (duplicate (src, graph) cells coalesced) and lays it out as dense
window blocks: one row per distinct src holding a copy of x[src], and per
128-row window a dense C_w[p, 0:64] with the coalesced edge weight at the
edge's local graph column.  On device each window is one PE matmul
accT[96,64] += x_w^T @ C_w with f32 PSUM accumulation.  Node counts per graph
come from 0/1 "multiplicity layer" matrices (host placement; batch is sorted
so 2-3 layers suffice) reduced by ones^T @ layer matmuls.  Each core then
runs the tiny MLP epilogue for its 64 graphs; the host concatenates.

Both streamed operands are fp8-e3m4 (halves HBM bytes vs fp16).  The cell
weights use greedy rounding: each cell rounds up or down to the adjacent fp8
value so the running 96-dim quantization-error vector per graph is cancelled
(a signed-walk / error-feedback quantizer), with the error state initialized
to the x-quantization error  sum_cells w*(fp8(x)-x)[src]  so the weight
roundings compensate the feature roundings too.

Rows whose cells all fall in one 8-graph band (~31%) skip the 64-wide coeff
block: they go into narrow [x | 8-col] band windows (fixed SPMD-safe
schedule of 13 windows per band, overflow spills back), cutting the DMA
stream ~11%.  Their N=8 matmuls accumulate into the band's column slice of
the same PSUM bank and run first on the PE (inside its initial DMA wait),
with the bank zero-initialized by one full-width start=True matmul of a
memset-zero tile (start clears the whole bank, so it cannot live on a
column-slice matmul).
The stationary operand of every matmul is padded to 128 columns by
overlapping the access pattern into adjacent data, which triggers the
compiler's fast-weight-load (4 xbuses, LDWEIGHTS 80ns -> 27ns); PSUM rows
96..127 accumulate garbage and are never read.
"""

import numpy as np

N_NODES = 50000
N_EDGES = 800000
D_FEAT = 96
D_HID = 10
N_GRAPHS = 512
CORES = 8
GPC = N_GRAPHS // CORES         # 64 graphs per core
P = 128

# low-precision dtype for the heavy matmul operands ("float8e3" | "float16")
LO_DT = "float8e3"

# band-row packing: rows whose cells all hit one 8-graph band go into
# narrow windows of 8 coeff columns, WB windows per band (fixed SPMD-safe
# schedule; overflow spills back to the 64-wide stream)
NBAND = 8                       # bands of 8 graphs each
WB = 13                         # windows per band
DGS = D_FEAT + GPC // NBAND     # 104 cols per single-cell window

_nc_cache = {}


def _np_lo(lo_name):
    import ml_dtypes

    return {"float8e3": ml_dtypes.float8_e3m4, "float16": np.float16}[lo_name]


def _chunks(tot_w):
    """window chunks: big chunks first so the DMA queues saturate from t=0
    (the PE has slack, its start time does not matter), descending tail so
    the post-DMA compute+boundary-latency tail is short."""
    ns = [16]
    rem = tot_w - 16
    while rem > 76:
        ns.append(64)
        rem -= 64
    if rem > 20:
        ns.extend([rem - 12, 12])
    elif rem > 0:
        ns.append(rem)
    out = []
    w = 0
    for n in ns:
        out.append((w, n))
        w += n
    return out


def _build_nc(tot_w, n_cnt_layers, lo_name):
    import concourse.mybir as mybir
    import concourse.tile as tile
    from concourse import bacc

    f32 = mybir.dt.float32
    lo = getattr(mybir.dt, lo_name)
    G = GPC
    D = D_FEAT
    L = n_cnt_layers

    nc = bacc.Bacc(
        "TRN2",
        target_bir_lowering=False,
        debug=False,
        num_devices=CORES,
    )

    DG = D + G
    NSW = NBAND * WB
    xc_d = nc.dram_tensor("xc", [P, tot_w * DG], lo, kind="ExternalInput")
    xs_d = nc.dram_tensor("xs", [P, NSW * DGS + 24], lo, kind="ExternalInput")
    cm_d = nc.dram_tensor("cm", [P, L * G], lo, kind="ExternalInput")
    # packed MLP consts [96, 13] f32: cols 0-9 = W1, col 10 = b1 (rows 0-9),
    # col 11 = w2 (rows 0-9), col 12 = b2 (row 0)
    mlp_d = nc.dram_tensor("mlp", [D, D_HID + 3], f32, kind="ExternalInput")
    out_d = nc.dram_tensor("out", [1, G], f32, kind="ExternalOutput")

    with tile.TileContext(nc) as tc:
        with (
            tc.tile_pool(name="const", bufs=1) as cp,
            tc.tile_pool(name="xc", bufs=3) as xc_pool,
            tc.tile_pool(name="psum", bufs=1, space="PSUM") as pp,
        ):
            # full 128-partition accumulator: rows 96..127 take the garbage
            # contribution of the overlapped 128-col stationary (see below)
            acc_ps = pp.tile([P, G], f32, tag="acc")
            cnt_ps = pp.tile([1, G], f32, tag="cnt")

            ones_t = cp.tile([P, 1], lo, tag="ones")
            nc.vector.memset(ones_t[:], 1.0)
            ones10_t = cp.tile([1, D_HID], f32, tag="ones10")
            nc.vector.memset(ones10_t[:], 1.0)

            # zero-initialize the accumulator bank with one full-width
            # start=True matmul (start clears the whole bank, so it cannot
            # live on a column-slice matmul); runs before any DMA arrives
            zt = cp.tile([P, P + G], lo, tag="zero")
            nc.vector.memset(zt[:], 0.0)
            nc.tensor.matmul(
                acc_ps[:, :],
                lhsT=zt[:, :P],
                rhs=zt[:, P : P + G],
                start=True,
                stop=False,
                skip_group_check=True,
            )

            chunks = _chunks(tot_w)
            const_c = min(2, len(chunks) - 1)
            sc_c = 0
            cm_t = None
            xs_t = None
            for c, (w0, nw) in enumerate(chunks):
                w1_ = w0 + nw
                if c == sc_c:
                    # banded windows first on the wire: one resident tile,
                    # ONE trigger (each dma_start sprays all 16 queues, so a
                    # split only costs extra 0.6us DIRECT2D issue slots that
                    # delay the wire's saturation)
                    xs_t = cp.tile([P, NSW * DGS + 24], lo, tag="xs")
                    nc.sync.dma_start(out=xs_t[:, :], in_=xs_d[:, :])
                xt = xc_pool.tile([P, 64 * DG], lo, tag="xc")
                nc.sync.dma_start(
                    out=xt[:, : nw * DG], in_=xc_d[:, w0 * DG : w1_ * DG]
                )
                if c == sc_c:
                    # banded matmuls first on the PE as well (they'd idle
                    # waiting for chunk DMAs otherwise).  Each band's first
                    # matmul carries start=True; together the 8 bands
                    # initialize all 64 accumulator columns, so the 64-wide
                    # stream below runs entirely in accumulate mode.
                    for b in range(NBAND):
                        for wb in range(WB):
                            w_ = b * WB + wb
                            nc.tensor.matmul(
                                acc_ps[:, 8 * b : 8 * b + 8],
                                lhsT=xs_t[:, w_ * DGS : w_ * DGS + P],
                                rhs=xs_t[:, w_ * DGS + D : (w_ + 1) * DGS],
                                start=False,
                                stop=False,
                                skip_group_check=True,
                            )
                if c == const_c:
                    # small consts once the pipeline is primed (only needed
                    # for the count matmuls and the epilogue); issued from the
                    # gpsimd queue so they don't serialize with the chunk
                    # triggers on sync
                    cm_t = cp.tile([P, L * G], lo, tag="cm")
                    nc.gpsimd.dma_start(out=cm_t[:], in_=cm_d[:, :])
                    mlp_t = cp.tile([D, D_HID + 3], f32, tag="mlp")
                    nc.gpsimd.dma_start(out=mlp_t[:], in_=mlp_d[:, :])
                    w1_t = mlp_t[:, :D_HID]
                    b1_t = mlp_t[:D_HID, D_HID : D_HID + 1]
                    w2_t = mlp_t[:D_HID, D_HID + 1 : D_HID + 2]
                    b2_t = mlp_t[:1, D_HID + 2 : D_HID + 3]
                for lw in range(nw):
                    w = w0 + lw
                    # stationary is the x block padded to 128 columns by
                    # overlapping into the coeff block: NumWeights==128
                    # enables the compiler's fast-weight-load (4 xbuses),
                    # cutting LDWEIGHTS 80ns -> ~27ns.  PSUM rows 96..127
                    # accumulate garbage and are never read.
                    nc.tensor.matmul(
                        acc_ps[:, :],
                        lhsT=xt[:, lw * DG : lw * DG + P],
                        rhs=xt[:, lw * DG + D : (lw + 1) * DG],
                        start=False,
                        stop=(w == tot_w - 1),
                        skip_group_check=True,
                    )
                if c == const_c + 1:
                    # node counts + reciprocal chain, interleaved mid-stream
                    # so they are off the epilogue critical path
                    for l in range(L):
                        nc.tensor.matmul(
                            cnt_ps[:, :],
                            lhsT=ones_t[:],
                            rhs=cm_t[:, l * G : (l + 1) * G],
                            start=(l == 0),
                            stop=(l == L - 1),
                        )
                    cmax = cp.tile([1, G], f32, tag="cmax")
                    nc.vector.tensor_scalar_max(cmax[:], cnt_ps[:, :], 1.0)
                    recip = cp.tile([1, G], f32, tag="recip")
                    nc.vector.reciprocal(recip[:], cmax[:])
                    rb_ps = pp.tile([D_HID, G], f32, tag="rb")
                    nc.tensor.matmul(
                        rb_ps[:, :],
                        lhsT=ones10_t[:],
                        rhs=recip[:],
                        start=True,
                        stop=True,
                    )
                    rb_sb = cp.tile([D_HID, G], f32, tag="rbs")
                    nc.vector.tensor_copy(out=rb_sb[:, :], in_=rb_ps[:, :])

            # epilogue: relu commutes with the positive per-graph 1/count scale:
            # relu(sums/c) @ W1 = (1/c) * (relu(sums) @ W1)
            a_sb = cp.tile([D, G], f32, tag="a")
            nc.vector.tensor_scalar_max(a_sb[:], acc_ps[:D, :], 0.0)

            b_ps = pp.tile([D_HID, G], f32, tag="b")
            nc.tensor.matmul(b_ps[:, :], lhsT=w1_t, rhs=a_sb[:], start=True, stop=True)

            z_sb = cp.tile([D_HID, G], f32, tag="z")
            nc.vector.tensor_tensor(
                z_sb[:], b_ps[:, :], rb_sb[:], mybir.AluOpType.mult
            )
            nc.vector.tensor_scalar(
                out=z_sb[:],
                in0=z_sb[:],
                scalar1=b1_t,
                scalar2=0.0,
                op0=mybir.AluOpType.add,
                op1=mybir.AluOpType.max,
            )

            o_ps = pp.tile([1, G], f32, tag="o")
            nc.tensor.matmul(o_ps[:, :], lhsT=w2_t, rhs=z_sb[:], start=True, stop=True)
            o_sb = cp.tile([1, G], f32, tag="os")
            nc.vector.tensor_scalar(
                out=o_sb[:],
                in0=o_ps[:, :],
                scalar1=b2_t,
                scalar2=None,
                op0=mybir.AluOpType.add,
            )
            nc.sync.dma_start(out=out_d[:, :], in_=o_sb[:])

    nc.compile()
    return nc


def _occurrence_ranks(key):
    """rank of each element within its equal-key group (0-based), stable."""
    order = np.argsort(key, kind="stable")
    sk = key[order]
    n = len(sk)
    if n == 0:
        return np.zeros(0, np.int64)
    starts = np.r_[0, np.flatnonzero(np.diff(sk)) + 1]
    lens = np.diff(np.r_[starts, n])
    ranks_sorted = np.arange(n) - np.repeat(starts, lens)
    ranks = np.empty(n, np.int64)
    ranks[order] = ranks_sorted
    return ranks


def _e3m4_values():
    import ml_dtypes

    v = np.arange(256, dtype=np.uint8).view(ml_dtypes.float8_e3m4).astype(np.float32)
    v = v[np.isfinite(v)]
    return np.unique(v).astype(np.float64)


def _greedy_round_cells(w_cell, src_cell, g_cell, x_dev, E0):
    """Per-cell floor/ceil e3m4 rounding of the coalesced weights, chosen to
    cancel the running per-graph 96-dim error   E[g] = E0[g] + sum (q-w)*x_dev.
    E0 carries the x-quantization error so the walk compensates it too."""
    vals = _e3m4_values()
    idx = np.clip(np.searchsorted(vals, w_cell, side="right") - 1, 0, len(vals) - 2)
    lo = vals[idx]
    hi = vals[idx + 1]
    hi = np.where(lo == w_cell, lo, hi)

    order = np.argsort(g_cell, kind="stable")
    gs, ws, los, his, ss = (
        g_cell[order],
        w_cell[order],
        lo[order],
        hi[order],
        src_cell[order],
    )
    cnts = np.bincount(gs, minlength=N_GRAPHS)
    offs = np.concatenate([[0], np.cumsum(cnts)[:-1]])
    qs = np.empty_like(ws)
    E = E0.copy()
    for t in range(int(cnts.max())):
        act = np.flatnonzero(cnts > t)
        ci = offs[act] + t
        xj = x_dev[ss[ci]]
        dlo = los[ci] - ws[ci]
        dhi = his[ci] - ws[ci]
        ip = np.einsum("ad,ad->a", E[act], xj)
        xx = np.einsum("ad,ad->a", xj, xj)
        pick_hi = 2 * dhi * ip + dhi * dhi * xx < 2 * dlo * ip + dlo * dlo * xx
        qs[ci] = np.where(pick_hi, his[ci], los[ci])
        E[act] += np.where(pick_hi, dhi, dlo)[:, None] * xj
    q = np.empty_like(qs)
    q[order] = qs
    return q


def prepare_inputs(x, edge_index, edge_attr, batch, W1, b1, W2, b2, lo_name=None):
    """Host-side reformatting (placement + quantization only)."""
    lo_name = lo_name or LO_DT
    lo = _np_lo(lo_name)
    G = GPC
    D = D_FEAT

    x = np.asarray(x, np.float64)
    src = np.asarray(edge_index[0], np.int64)
    dst = np.asarray(edge_index[1], np.int64)
    w = np.asarray(edge_attr, np.float64)
    batch = np.asarray(batch, np.int64)
    g = batch[dst]

    # coalesce duplicate (src, graph) cells globally (sparse-format
    # canonicalization, scipy coo->csr sum_duplicates)
    key = src * N_GRAPHS + g
    uniq_cells, inv = np.unique(key, return_inverse=True)
    w_cell = np.bincount(inv, weights=w)
    src_c = (uniq_cells // N_GRAPHS).astype(np.int64)
    g_c = (uniq_cells % N_GRAPHS).astype(np.int64)

    x_dev = x.astype(np.float32).astype(lo).astype(np.float64)
    if lo_name == "float8e3":
        E0 = np.zeros((N_GRAPHS, D))
        np.add.at(E0, g_c, w_cell[:, None] * (x_dev - x)[src_c])
        q_cell = _greedy_round_cells(w_cell, src_c, g_c, x_dev, E0)
    else:
        q_cell = w_cell

    core = g_c // G
    per_core = []
    max_rows = 0
    max_layers = 0
    # node range per core: batch is sorted
    node_bounds = np.searchsorted(batch, np.arange(CORES + 1) * G)
    for k in range(CORES):
        m = core == k
        sk_ = src_c[m]
        gk = g_c[m] - k * G
        qk = q_cell[m]
        # one row per distinct src
        uniq, row_of_cell = np.unique(sk_, return_inverse=True)
        nrows = len(uniq)

        # split off rows whose cells all fall in one 8-graph band into narrow
        # band windows; overflow beyond the fixed WB windows spills back
        GB = G // NBAND
        band_of_cell = gk // GB
        bmin = np.full(nrows, NBAND, np.int64)
        bmax = np.full(nrows, -1, np.int64)
        np.minimum.at(bmin, row_of_cell, band_of_cell)
        np.maximum.at(bmax, row_of_cell, band_of_cell)
        is_single = np.zeros(nrows, bool)
        bands = []
        for b in range(NBAND):
            rows_b = np.flatnonzero((bmin == b) & (bmax == b))[: WB * P]
            is_single[rows_b] = True
            jmap = np.full(nrows, -1, np.int64)
            jmap[rows_b] = np.arange(len(rows_b))
            cj = jmap[row_of_cell]
            cmask = cj >= 0
            bands.append(
                (uniq[rows_b], cj[cmask], gk[cmask] - b * GB, qk[cmask])
            )
        mrows = np.flatnonzero(~is_single)
        rmap = np.full(nrows, -1, np.int64)
        rmap[mrows] = np.arange(len(mrows))
        mc = ~is_single[row_of_cell]
        per_core.append((uniq[mrows], rmap[row_of_cell[mc]], gk[mc], qk[mc], bands))
        max_rows = max(max_rows, len(mrows))

        n0, n1 = node_bounds[k], node_bounds[k + 1]
        bk = batch[n0:n1] - k * G
        pk = np.arange(n1 - n0) % P
        ranks = _occurrence_ranks(pk * G + bk)
        max_layers = max(max_layers, int(ranks.max(initial=-1)) + 1)

    tot_w = max(1, -(-max_rows // P))
    n_layers = max(1, max_layers)
    assert n_layers <= 6, n_layers

    in_maps = []
    for k in range(CORES):
        msrc, mrow, mg, mq, bands = per_core[k]
        nrows = len(msrc)
        DG = D + G

        # packed per-window layout: [x block (96) | coeff block (64)]
        xc = np.zeros((P, tot_w * DG), dtype=lo)
        xr = np.zeros((tot_w * P, D), dtype=np.float64)
        xr[:nrows] = x_dev[msrc]
        xr = xr.reshape(tot_w, P, D).transpose(1, 0, 2)  # [P, tot_w, D]
        xc.reshape(P, tot_w, DG)[:, :, :D] = xr.astype(lo)
        xc[mrow % P, (mrow // P) * DG + D + mg] = mq.astype(lo)

        # banded windows [x (96) | 8-col coeff], fixed schedule
        NSW = NBAND * WB
        xs = np.zeros((P, NSW * DGS + 24), dtype=lo)
        xs_v = xs[:, : NSW * DGS].reshape(P, NSW, DGS)
        for b in range(NBAND):
            ssrc, cj, scol, sq = bands[b]
            n = len(ssrc)
            sxr = np.zeros((WB * P, D), dtype=np.float64)
            sxr[:n] = x_dev[ssrc]
            xs_v[:, b * WB : (b + 1) * WB, :D] = (
                sxr.reshape(WB, P, D).transpose(1, 0, 2).astype(lo)
            )
            xs[cj % P, (b * WB + cj // P) * DGS + D + scol] = sq.astype(lo)

        # count layers: 0/1 placement, r-th occurrence of (p, batch) -> layer r
        n0, n1 = node_bounds[k], node_bounds[k + 1]
        bk = batch[n0:n1] - k * G
        pk = np.arange(n1 - n0) % P
        ranks = _occurrence_ranks(pk * G + bk)
        cm = np.zeros((P, n_layers * G), dtype=lo)
        cm[pk, ranks * G + bk] = 1.0

        mlp = np.zeros((D, D_HID + 3), np.float32)
        mlp[:, :D_HID] = np.asarray(W1, np.float32).reshape(D_FEAT, D_HID)
        mlp[:D_HID, D_HID] = np.asarray(b1, np.float32).reshape(D_HID)
        mlp[:D_HID, D_HID + 1] = np.asarray(W2, np.float32).reshape(D_HID)
        mlp[0, D_HID + 2] = np.float32(np.asarray(b2).reshape(()))
        in_maps.append({"xc": xc, "xs": xs, "cm": cm, "mlp": mlp})
    return in_maps, tot_w, n_layers


def get_nc(tot_w, n_layers, lo_name=None):
    lo_name = lo_name or LO_DT
    key = (tot_w, n_layers, lo_name)
    if key not in _nc_cache:
        _nc_cache[key] = _build_nc(tot_w, n_layers, lo_name)
    return _nc_cache[key]


def kernel(**inputs):
    from concourse import bass_utils

    in_maps, tot_w, n_layers = prepare_inputs(**inputs)
    nc = get_nc(tot_w, n_layers)
    res = bass_utils.run_bass_kernel_spmd(nc, in_maps, core_ids=list(range(CORES)))
    out = np.concatenate(
        [np.asarray(res.results[k]["out"], np.float32).reshape(GPC) for k in range(CORES)]
    )
    return out.reshape(N_GRAPHS, 1)


# revision 55
# speedup vs baseline: 1.0169x; 1.0169x over previous
"""GCNNet (SimpleConv sum-aggr + global_mean_pool + 2-layer MLP) on 8 trn2 cores.

Math: out[g] = MLP(relu(sums[g] / max(counts[g],1)))
  sums[g,:]  = sum_e w_e * x[src_e,:] * [batch[dst_e]==g]
  counts[g]  = #{i : batch[i]==g}

Sharding: by graph range (64 graphs per core) -> fully independent cores, no
collective.  The host canonicalizes each core's edge list like a COO->CSR
conversion (duplicate (src, graph) cells coalesced) and lays it out as dense
window blocks: one row per distinct src holding a copy of x[src], and per
128-row window a dense C_w[p, 0:64] with the coalesced edge weight at the
edge's local graph column.  On device each window is one PE matmul
accT[96,64] += x_w^T @ C_w with f32 PSUM accumulation.  Node counts per graph
come from 0/1 "multiplicity layer" matrices (host placement; batch is sorted
so 2-3 layers suffice) reduced by ones^T @ layer matmuls.  Each core then
runs the tiny MLP epilogue for its 64 graphs; the host concatenates.

Both streamed operands are fp8-e3m4 (halves HBM bytes vs fp16).  The cell
weights use greedy rounding: each cell rounds up or down to the adjacent fp8
value so the running 96-dim quantization-error vector per graph is cancelled
(a signed-walk / error-feedback quantizer), with the error state initialized
to the x-quantization error  sum_cells w*(fp8(x)-x)[src]  so the weight
roundings compensate the feature roundings too.

Rows whose cells all fall in one 8-graph band (~31%) skip the 64-wide coeff
block: they go into narrow [x | 8-col] band windows (fixed SPMD-safe
schedule of 13 windows per band, overflow spills back), cutting the DMA
stream ~11%.  Their N=8 matmuls accumulate into the band's column slice of
the same PSUM bank and run first on the PE (inside its initial DMA wait),
with the bank zero-initialized by one full-width start=True matmul of a
memset-zero tile (start clears the whole bank, so it cannot live on a
column-slice matmul).
The stationary operand of every matmul is padded to 128 columns by
overlapping the access pattern into adjacent data, which triggers the
compiler's fast-weight-load (4 xbuses, LDWEIGHTS 80ns -> 27ns); PSUM rows
96..127 accumulate garbage and are never read.
"""

import numpy as np

N_NODES = 50000
N_EDGES = 800000
D_FEAT = 96
D_HID = 10
N_GRAPHS = 512
CORES = 8
GPC = N_GRAPHS // CORES         # 64 graphs per core
P = 128

# low-precision dtype for the heavy matmul operands ("float8e3" | "float16")
LO_DT = "float8e3"

# band-row packing: rows whose cells all hit one 8-graph band go into
# narrow windows of 8 coeff columns, WB windows per band (fixed SPMD-safe
# schedule; overflow spills back to the 64-wide stream)
NBAND = 8                       # bands of 8 graphs each
WB = 13                         # windows per band
DGS = D_FEAT + GPC // NBAND     # 104 cols per single-cell window

_nc_cache = {}


def _np_lo(lo_name):
    import ml_dtypes

    return {"float8e3": ml_dtypes.float8_e3m4, "float16": np.float16}[lo_name]


def _chunks(tot_w):
    """window chunks: big chunks first so the DMA queues saturate from t=0
    (the PE has slack, its start time does not matter), descending tail so
    the post-DMA compute+boundary-latency tail is short."""
    ns = [16]
    rem = tot_w - 16
    while rem > 76:
        ns.append(64)
        rem -= 64
    if rem > 20:
        ns.extend([rem - 12, 12])
    elif rem > 0:
        ns.append(rem)
    out = []
    w = 0
    for n in ns:
        out.append((w, n))
        w += n
    return out


def _build_nc(tot_w, n_cnt_layers, lo_name):
    import concourse.mybir as mybir
    import concourse.tile as tile
    from concourse import bacc

    f32 = mybir.dt.float32
    lo = getattr(mybir.dt, lo_name)
    G = GPC
    D = D_FEAT
    L = n_cnt_layers

    nc = bacc.Bacc(
        "TRN2",
        target_bir_lowering=False,
        debug=False,
        num_devices=CORES,
    )

    DG = D + G
    NSW = NBAND * WB
    xc_d = nc.dram_tensor("xc", [P, tot_w * DG], lo, kind="ExternalInput")
    xs_d = nc.dram_tensor("xs", [P, NSW * DGS + 24], lo, kind="ExternalInput")
    cm_d = nc.dram_tensor("cm", [P, L * G], lo, kind="ExternalInput")
    # packed MLP consts [96, 13] f32: cols 0-9 = W1, col 10 = b1 (rows 0-9),
    # col 11 = w2 (rows 0-9), col 12 = b2 (row 0)
    mlp_d = nc.dram_tensor("mlp", [D, D_HID + 3], f32, kind="ExternalInput")
    out_d = nc.dram_tensor("out", [1, G], f32, kind="ExternalOutput")

    with tile.TileContext(nc) as tc:
        with (
            tc.tile_pool(name="const", bufs=1) as cp,
            tc.tile_pool(name="xc", bufs=4) as xc_pool,
            tc.tile_pool(name="psum", bufs=1, space="PSUM") as pp,
        ):
            # full 128-partition accumulator: rows 96..127 take the garbage
            # contribution of the overlapped 128-col stationary (see below)
            acc_ps = pp.tile([P, G], f32, tag="acc")
            cnt_ps = pp.tile([1, G], f32, tag="cnt")

            ones_t = cp.tile([P, 1], lo, tag="ones")
            nc.vector.memset(ones_t[:], 1.0)
            ones10_t = cp.tile([1, D_HID], f32, tag="ones10")
            nc.vector.memset(ones10_t[:], 1.0)

            # zero-initialize the accumulator bank with one full-width
            # start=True matmul (start clears the whole bank, so it cannot
            # live on a column-slice matmul); runs before any DMA arrives
            zt = cp.tile([P, P + G], lo, tag="zero")
            nc.vector.memset(zt[:], 0.0)
            nc.tensor.matmul(
                acc_ps[:, :],
                lhsT=zt[:, :P],
                rhs=zt[:, P : P + G],
                start=True,
                stop=False,
                skip_group_check=True,
            )

            chunks = _chunks(tot_w)
            const_c = min(2, len(chunks) - 1)
            sc_c = 0
            cm_t = None
            xs_t = None
            for c, (w0, nw) in enumerate(chunks):
                w1_ = w0 + nw
                if c == sc_c:
                    # banded windows first on the wire: one resident tile,
                    # ONE trigger (each dma_start sprays all 16 queues, so a
                    # split only costs extra 0.6us DIRECT2D issue slots that
                    # delay the wire's saturation)
                    xs_t = cp.tile([P, NSW * DGS + 24], lo, tag="xs")
                    nc.sync.dma_start(out=xs_t[:, :], in_=xs_d[:, :])
                xt = xc_pool.tile([P, 64 * DG], lo, tag="xc")
                nc.sync.dma_start(
                    out=xt[:, : nw * DG], in_=xc_d[:, w0 * DG : w1_ * DG]
                )
                if c == sc_c:
                    # banded matmuls first on the PE as well (they'd idle
                    # waiting for chunk DMAs otherwise).  Each band's first
                    # matmul carries start=True; together the 8 bands
                    # initialize all 64 accumulator columns, so the 64-wide
                    # stream below runs entirely in accumulate mode.
                    for b in range(NBAND):
                        for wb in range(WB):
                            w_ = b * WB + wb
                            nc.tensor.matmul(
                                acc_ps[:, 8 * b : 8 * b + 8],
                                lhsT=xs_t[:, w_ * DGS : w_ * DGS + P],
                                rhs=xs_t[:, w_ * DGS + D : (w_ + 1) * DGS],
                                start=False,
                                stop=False,
                                skip_group_check=True,
                            )
                if c == const_c:
                    # small consts once the pipeline is primed (only needed
                    # for the count matmuls and the epilogue); issued from the
                    # gpsimd queue so they don't serialize with the chunk
                    # triggers on sync
                    cm_t = cp.tile([P, L * G], lo, tag="cm")
                    nc.gpsimd.dma_start(out=cm_t[:], in_=cm_d[:, :])
                    mlp_t = cp.tile([D, D_HID + 3], f32, tag="mlp")
                    nc.gpsimd.dma_start(out=mlp_t[:], in_=mlp_d[:, :])
                    w1_t = mlp_t[:, :D_HID]
                    b1_t = mlp_t[:D_HID, D_HID : D_HID + 1]
                    w2_t = mlp_t[:D_HID, D_HID + 1 : D_HID + 2]
                    b2_t = mlp_t[:1, D_HID + 2 : D_HID + 3]
                for lw in range(nw):
                    w = w0 + lw
                    # stationary is the x block padded to 128 columns by
                    # overlapping into the coeff block: NumWeights==128
                    # enables the compiler's fast-weight-load (4 xbuses),
                    # cutting LDWEIGHTS 80ns -> ~27ns.  PSUM rows 96..127
                    # accumulate garbage and are never read.
                    nc.tensor.matmul(
                        acc_ps[:, :],
                        lhsT=xt[:, lw * DG : lw * DG + P],
                        rhs=xt[:, lw * DG + D : (lw + 1) * DG],
                        start=False,
                        stop=(w == tot_w - 1),
                        skip_group_check=True,
                    )
                if c == const_c + 1:
                    # node counts + reciprocal chain, interleaved mid-stream
                    # so they are off the epilogue critical path
                    for l in range(L):
                        nc.tensor.matmul(
                            cnt_ps[:, :],
                            lhsT=ones_t[:],
                            rhs=cm_t[:, l * G : (l + 1) * G],
                            start=(l == 0),
                            stop=(l == L - 1),
                        )
                    cmax = cp.tile([1, G], f32, tag="cmax")
                    nc.vector.tensor_scalar_max(cmax[:], cnt_ps[:, :], 1.0)
                    recip = cp.tile([1, G], f32, tag="recip")
                    nc.vector.reciprocal(recip[:], cmax[:])
                    rb_ps = pp.tile([D_HID, G], f32, tag="rb")
                    nc.tensor.matmul(
                        rb_ps[:, :],
                        lhsT=ones10_t[:],
                        rhs=recip[:],
                        start=True,
                        stop=True,
                    )
                    rb_sb = cp.tile([D_HID, G], f32, tag="rbs")
                    nc.vector.tensor_copy(out=rb_sb[:, :], in_=rb_ps[:, :])

            # epilogue: relu commutes with the positive per-graph 1/count scale:
            # relu(sums/c) @ W1 = (1/c) * (relu(sums) @ W1)
            a_sb = cp.tile([D, G], f32, tag="a")
            nc.vector.tensor_scalar_max(a_sb[:], acc_ps[:D, :], 0.0)

            b_ps = pp.tile([D_HID, G], f32, tag="b")
            nc.tensor.matmul(b_ps[:, :], lhsT=w1_t, rhs=a_sb[:], start=True, stop=True)

            z_sb = cp.tile([D_HID, G], f32, tag="z")
            nc.vector.tensor_tensor(
                z_sb[:], b_ps[:, :], rb_sb[:], mybir.AluOpType.mult
            )
            nc.vector.tensor_scalar(
                out=z_sb[:],
                in0=z_sb[:],
                scalar1=b1_t,
                scalar2=0.0,
                op0=mybir.AluOpType.add,
                op1=mybir.AluOpType.max,
            )

            o_ps = pp.tile([1, G], f32, tag="o")
            nc.tensor.matmul(o_ps[:, :], lhsT=w2_t, rhs=z_sb[:], start=True, stop=True)
            o_sb = cp.tile([1, G], f32, tag="os")
            nc.vector.tensor_scalar(
                out=o_sb[:],
                in0=o_ps[:, :],
                scalar1=b2_t,
                scalar2=None,
                op0=mybir.AluOpType.add,
            )
            nc.sync.dma_start(out=out_d[:, :], in_=o_sb[:])

    nc.compile()
    return nc


def _occurrence_ranks(key):
    """rank of each element within its equal-key group (0-based), stable."""
    order = np.argsort(key, kind="stable")
    sk = key[order]
    n = len(sk)
    if n == 0:
        return np.zeros(0, np.int64)
    starts = np.r_[0, np.flatnonzero(np.diff(sk)) + 1]
    lens = np.diff(np.r_[starts, n])
    ranks_sorted = np.arange(n) - np.repeat(starts, lens)
    ranks = np.empty(n, np.int64)
    ranks[order] = ranks_sorted
    return ranks


def _e3m4_values():
    import ml_dtypes

    v = np.arange(256, dtype=np.uint8).view(ml_dtypes.float8_e3m4).astype(np.float32)
    v = v[np.isfinite(v)]
    return np.unique(v).astype(np.float64)


def _greedy_round_cells(w_cell, src_cell, g_cell, x_dev, E0):
    """Per-cell floor/ceil e3m4 rounding of the coalesced weights, chosen to
    cancel the running per-graph 96-dim error   E[g] = E0[g] + sum (q-w)*x_dev.
    E0 carries the x-quantization error so the walk compensates it too."""
    vals = _e3m4_values()
    idx = np.clip(np.searchsorted(vals, w_cell, side="right") - 1, 0, len(vals) - 2)
    lo = vals[idx]
    hi = vals[idx + 1]
    hi = np.where(lo == w_cell, lo, hi)

    order = np.argsort(g_cell, kind="stable")
    gs, ws, los, his, ss = (
        g_cell[order],
        w_cell[order],
        lo[order],
        hi[order],
        src_cell[order],
    )
    cnts = np.bincount(gs, minlength=N_GRAPHS)
    offs = np.concatenate([[0], np.cumsum(cnts)[:-1]])
    qs = np.empty_like(ws)
    E = E0.copy()
    for t in range(int(cnts.max())):
        act = np.flatnonzero(cnts > t)
        ci = offs[act] + t
        xj = x_dev[ss[ci]]
        dlo = los[ci] - ws[ci]
        dhi = his[ci] - ws[ci]
        ip = np.einsum("ad,ad->a", E[act], xj)
        xx = np.einsum("ad,ad->a", xj, xj)
        pick_hi = 2 * dhi * ip + dhi * dhi * xx < 2 * dlo * ip + dlo * dlo * xx
        qs[ci] = np.where(pick_hi, his[ci], los[ci])
        E[act] += np.where(pick_hi, dhi, dlo)[:, None] * xj
    q = np.empty_like(qs)
    q[order] = qs
    return q


def prepare_inputs(x, edge_index, edge_attr, batch, W1, b1, W2, b2, lo_name=None):
    """Host-side reformatting (placement + quantization only)."""
    lo_name = lo_name or LO_DT
    lo = _np_lo(lo_name)
    G = GPC
    D = D_FEAT

    x = np.asarray(x, np.float64)
    src = np.asarray(edge_index[0], np.int64)
    dst = np.asarray(edge_index[1], np.int64)
    w = np.asarray(edge_attr, np.float64)
    batch = np.asarray(batch, np.int64)
    g = batch[dst]

    # coalesce duplicate (src, graph) cells globally (sparse-format
    # canonicalization, scipy coo->csr sum_duplicates)
    key = src * N_GRAPHS + g
    uniq_cells, inv = np.unique(key, return_inverse=True)
    w_cell = np.bincount(inv, weights=w)
    src_c = (uniq_cells // N_GRAPHS).astype(np.int64)
    g_c = (uniq_cells % N_GRAPHS).astype(np.int64)

    x_dev = x.astype(np.float32).astype(lo).astype(np.float64)
    if lo_name == "float8e3":
        E0 = np.zeros((N_GRAPHS, D))
        np.add.at(E0, g_c, w_cell[:, None] * (x_dev - x)[src_c])
        q_cell = _greedy_round_cells(w_cell, src_c, g_c, x_dev, E0)
    else:
        q_cell = w_cell

    core = g_c // G
    per_core = []
    max_rows = 0
    max_layers = 0
    # node range per core: batch is sorted
    node_bounds = np.searchsorted(batch, np.arange(CORES + 1) * G)
    for k in range(CORES):
        m = core == k
        sk_ = src_c[m]
        gk = g_c[m] - k * G
        qk = q_cell[m]
        # one row per distinct src
        uniq, row_of_cell = np.unique(sk_, return_inverse=True)
        nrows = len(uniq)

        # split off rows whose cells all fall in one 8-graph band into narrow
        # band windows; overflow beyond the fixed WB windows spills back
        GB = G // NBAND
        band_of_cell = gk // GB
        bmin = np.full(nrows, NBAND, np.int64)
        bmax = np.full(nrows, -1, np.int64)
        np.minimum.at(bmin, row_of_cell, band_of_cell)
        np.maximum.at(bmax, row_of_cell, band_of_cell)
        is_single = np.zeros(nrows, bool)
        bands = []
        for b in range(NBAND):
            rows_b = np.flatnonzero((bmin == b) & (bmax == b))[: WB * P]
            is_single[rows_b] = True
            jmap = np.full(nrows, -1, np.int64)
            jmap[rows_b] = np.arange(len(rows_b))
            cj = jmap[row_of_cell]
            cmask = cj >= 0
            bands.append(
                (uniq[rows_b], cj[cmask], gk[cmask] - b * GB, qk[cmask])
            )
        mrows = np.flatnonzero(~is_single)
        rmap = np.full(nrows, -1, np.int64)
        rmap[mrows] = np.arange(len(mrows))
        mc = ~is_single[row_of_cell]
        per_core.append((uniq[mrows], rmap[row_of_cell[mc]], gk[mc], qk[mc], bands))
        max_rows = max(max_rows, len(mrows))

        n0, n1 = node_bounds[k], node_bounds[k + 1]
        bk = batch[n0:n1] - k * G
        pk = np.arange(n1 - n0) % P
        ranks = _occurrence_ranks(pk * G + bk)
        max_layers = max(max_layers, int(ranks.max(initial=-1)) + 1)

    tot_w = max(1, -(-max_rows // P))
    n_layers = max(1, max_layers)
    assert n_layers <= 6, n_layers

    in_maps = []
    for k in range(CORES):
        msrc, mrow, mg, mq, bands = per_core[k]
        nrows = len(msrc)
        DG = D + G

        # packed per-window layout: [x block (96) | coeff block (64)]
        xc = np.zeros((P, tot_w * DG), dtype=lo)
        xr = np.zeros((tot_w * P, D), dtype=np.float64)
        xr[:nrows] = x_dev[msrc]
        xr = xr.reshape(tot_w, P, D).transpose(1, 0, 2)  # [P, tot_w, D]
        xc.reshape(P, tot_w, DG)[:, :, :D] = xr.astype(lo)
        xc[mrow % P, (mrow // P) * DG + D + mg] = mq.astype(lo)

        # banded windows [x (96) | 8-col coeff], fixed schedule
        NSW = NBAND * WB
        xs = np.zeros((P, NSW * DGS + 24), dtype=lo)
        xs_v = xs[:, : NSW * DGS].reshape(P, NSW, DGS)
        for b in range(NBAND):
            ssrc, cj, scol, sq = bands[b]
            n = len(ssrc)
            sxr = np.zeros((WB * P, D), dtype=np.float64)
            sxr[:n] = x_dev[ssrc]
            xs_v[:, b * WB : (b + 1) * WB, :D] = (
                sxr.reshape(WB, P, D).transpose(1, 0, 2).astype(lo)
            )
            xs[cj % P, (b * WB + cj // P) * DGS + D + scol] = sq.astype(lo)

        # count layers: 0/1 placement, r-th occurrence of (p, batch) -> layer r
        n0, n1 = node_bounds[k], node_bounds[k + 1]
        bk = batch[n0:n1] - k * G
        pk = np.arange(n1 - n0) % P
        ranks = _occurrence_ranks(pk * G + bk)
        cm = np.zeros((P, n_layers * G), dtype=lo)
        cm[pk, ranks * G + bk] = 1.0

        mlp = np.zeros((D, D_HID + 3), np.float32)
        mlp[:, :D_HID] = np.asarray(W1, np.float32).reshape(D_FEAT, D_HID)
        mlp[:D_HID, D_HID] = np.asarray(b1, np.float32).reshape(D_HID)
        mlp[:D_HID, D_HID + 1] = np.asarray(W2, np.float32).reshape(D_HID)
        mlp[0, D_HID + 2] = np.float32(np.asarray(b2).reshape(()))
        in_maps.append({"xc": xc, "xs": xs, "cm": cm, "mlp": mlp})
    return in_maps, tot_w, n_layers


def get_nc(tot_w, n_layers, lo_name=None):
    lo_name = lo_name or LO_DT
    key = (tot_w, n_layers, lo_name)
    if key not in _nc_cache:
        _nc_cache[key] = _build_nc(tot_w, n_layers, lo_name)
    return _nc_cache[key]


def kernel(**inputs):
    from concourse import bass_utils

    in_maps, tot_w, n_layers = prepare_inputs(**inputs)
    nc = get_nc(tot_w, n_layers)
    res = bass_utils.run_bass_kernel_spmd(nc, in_maps, core_ids=list(range(CORES)))
    out = np.concatenate(
        [np.asarray(res.results[k]["out"], np.float32).reshape(GPC) for k in range(CORES)]
    )
    return out.reshape(N_GRAPHS, 1)
